# revision 21
# baseline (speedup 1.0000x reference)
"""Darknet 3x3 conv block (conv * mask + bias) on 8 TRN2 NeuronCores.

Problem: x[1,512,192,192] (*) w[512,512,3,3] stride1 pad1, then *mask + bias.

Strategy (masked gather-GEMM):
  The reference is conv(x,w)*mask + b: output pixels where mask==0 are
  exactly b, so only ~50% of output pixels need the conv.  The host
  gathers im2col columns for the mask==1 pixels only and splits them
  evenly across the 8 cores; each core runs a dense GEMM over its
  pixel list and the host scatters results back (mask==0 filled with b).
  This halves device FLOPs vs the dense conv.

  - Host packs, per core: xg [128, sum_c CC*TAPS*csz_c] bf16 - the
    gathered im2col columns, chunk-major ([chunk][cc][tap][px], chunks
    of <=512 px to match one PSUM bank).  Weights [c128, FM, CC, TAP,
    f128] bf16 (replicated).  Bias [128, FM] f32.
  - Device: per chunk, per fm: accumulate CC*TAPS=36 matmuls
    (lhsT = w tile [c128,f128], rhs = gathered xg [c128, csz]) into one
    PSUM bank, ScalarE bias-add epilogue, DMA out.  No mask multiply on
    device (all gathered pixels have mask==1).
  - DMA: x on the SP ring (chunk0 split in 4 cc pieces for a fast head,
    then whole chunks, double-buffered); w pieces + bias + y on ACT.
  - Host unshard: scatter [512, npx] core outputs into b-filled output.
"""

import sys

for _p in ("/opt/trn_rl_repo",):
    if _p not in sys.path:
        sys.path.insert(0, _p)

import numpy as np
import ml_dtypes

N_CORES = 8
C = 512
F = 512
H = 192
W = 192
K = 3
HP = H + 2                 # padded spatial
WP = W + 2
CC = C // 128              # c chunks = 4
FM = F // 128              # f chunks = 4
TAPS = K * K
CHUNK = 512                # px per PSUM bank (2KB of f32)
NWARM = 8                  # PE warmup matmuls while first DMAs land

_CACHE = {}


def _chunks(npx):
    """Descending chunk sizes (big first -> long fm0 window covers the
    head DMA; small remainder last -> short tail)."""
    rem = npx % CHUNK
    return [CHUNK] * (npx // CHUNK) + ([rem] if rem else [])


def _build(npx):
    import concourse.bacc as bacc
    import concourse.mybir as mybir
    from concourse.tile import TileContext

    BF = mybir.dt.bfloat16
    F32 = mybir.dt.float32

    chunks = _chunks(npx)
    xg_cols = CC * TAPS * npx

    nc = bacc.Bacc(trn_type="TRN2", num_devices=N_CORES)
    xg_sh = nc.dram_tensor("xg_sh", [128, xg_cols], BF, kind="ExternalInput")
    w_sh = nc.dram_tensor("w_sh", [128, FM, CC, TAPS, 128], BF, kind="ExternalInput")
    b_sh = nc.dram_tensor("b_sh", [128, FM], F32, kind="ExternalInput")
    y_sh = nc.dram_tensor("y_sh", [FM, 128, npx], F32, kind="ExternalOutput")

    with TileContext(nc) as tc:
        with (
            tc.tile_pool(name="const", bufs=1) as cpool,
            tc.tile_pool(name="pwarm", bufs=1, space="PSUM") as wpool,
            tc.tile_pool(name="psum", bufs=7, space="PSUM") as ppool,
            tc.tile_pool(name="outp", bufs=4) as opool,
            tc.tile_pool(name="xin", bufs=3) as xpool,
        ):
            # PE warmup while the first DMAs land (HAM pre-warm + head fill)
            scratch = cpool.tile([128, CHUNK], BF)
            nc.vector.memset(scratch[:], 0.0)
            dps = wpool.tile([128, CHUNK], F32, name="dps", tag="warm")
            for _ in range(NWARM):
                nc.tensor.matmul(dps[:], scratch[:, :128], scratch[:],
                                 start=True, stop=True)

            wt = cpool.tile([128, FM, CC, TAPS, 128], BF)
            bt = cpool.tile([128, FM], F32)

            # --- DMA issue, first-use order ------------------------------
            # x rides SP (chunk0 as 4 cc pieces, later chunks whole); w
            # rides ACT as 16 per-(fm,cc) pieces + bias, then the y
            # outputs.  The many small w pieces deliberately trickle
            # (issue-serialized) - consolidating them makes w complete
            # sooner but steals early HBM bandwidth from the critical x
            # stream (measured net loss).  gpsimd DMA (software DGE,
            # ~25GB/s) is too slow for any of this.
            xts = [xpool.tile([128, CC * TAPS * CHUNK], BF,
                              name=f"x{ci}", tag="x")
                   for ci in range(len(chunks))]
            piece = TAPS * chunks[0]
            for cc in range(CC):
                nc.sync.dma_start(out=xts[0][:, cc * piece:(cc + 1) * piece],
                                  in_=xg_sh[:, cc * piece:(cc + 1) * piece])
            for cc in range(CC):
                nc.scalar.dma_start(out=wt[:, 0, cc], in_=w_sh[:, 0, cc])
            nc.scalar.dma_start(out=bt[:], in_=b_sh[:])
            for fm in range(1, FM):
                for cc in range(CC):
                    nc.scalar.dma_start(out=wt[:, fm, cc], in_=w_sh[:, fm, cc])
            c_off = CC * TAPS * chunks[0]
            for ci, csz in enumerate(chunks[1:], start=1):
                n = CC * TAPS * csz
                nc.sync.dma_start(out=xts[ci][:, :n],
                                  in_=xg_sh[:, c_off:c_off + n])
                c_off += n

            # --- main loop ----------------------------------------------
            # Sequential accumulation chains into one PSUM bank per
            # (chunk, fm): back-to-back matmuls into the same bank stream
            # at the full 2.4GHz PE rate.  (Keeping several accumulation
            # banks open concurrently - fm-interleaved or cc-major orders -
            # was measured to throttle the PE ~20% for the whole run.)
            px0 = 0
            for ci, csz in enumerate(chunks):
                xv = xts[ci][:, :CC * TAPS * csz].rearrange(
                    "p (c t q) -> p c t q", c=CC, t=TAPS)
                for fm in range(FM):
                    pt = ppool.tile([128, CHUNK], F32,
                                    name=f"ps_{ci}_{fm}", tag="ps")
                    for a in range(CC * TAPS):
                        cc, o = divmod(a, TAPS)
                        nc.tensor.matmul(
                            pt[:, :csz], wt[:, fm, cc, o], xv[:, cc, o],
                            start=(a == 0), stop=(a == CC * TAPS - 1),
                        )
                    ot = opool.tile([128, CHUNK], F32,
                                    name=f"ot_{ci}_{fm}", tag="ot")
                    nc.scalar.activation(
                        ot[:, :csz], pt[:, :csz],
                        mybir.ActivationFunctionType.Identity,
                        bias=bt[:, fm:fm + 1],
                    )
                    nc.scalar.dma_start(out=y_sh[fm, :, px0:px0 + csz],
                                        in_=ot[:, :csz])
                px0 += csz

    nc.compile()
    return nc


def _pack(x, w, b, mask, npx, chunks):
    x = np.asarray(x, dtype=np.float32)
    w = np.asarray(w, dtype=np.float32)
    b = np.asarray(b, dtype=np.float32)
    mask = np.asarray(mask)

    xp = np.zeros((C, HP, WP), dtype=np.float32)
    xp[:, 1:-1, 1:-1] = x[0]
    xpb = xp.astype(ml_dtypes.bfloat16).reshape(CC, 128, HP * WP)

    # [kh,kw,c,f] -> [tap, cc, c_local, fm, f128] -> [c_local, fm, cc, tap, f128]
    wt = w.transpose(2, 3, 1, 0).reshape(TAPS, CC, 128, FM, 128)
    wt = np.ascontiguousarray(wt.transpose(2, 3, 1, 0, 4)).astype(ml_dtypes.bfloat16)
    b_re = np.ascontiguousarray(b.reshape(FM, 128).T)

    hs, ws = np.nonzero(mask)
    cnt = len(hs)
    total = npx * N_CORES
    # top-left of each 3x3 window in the padded image (output px (h,w)
    # reads padded rows h..h+2); pad with a repeat of the last real
    # coordinate (its duplicate output scatters the same value).
    base = hs.astype(np.int64) * WP + ws.astype(np.int64)
    if cnt == 0:
        base_pad = np.zeros(total, dtype=np.int64)
    else:
        base_pad = np.concatenate(
            [base, np.full(total - cnt, base[-1], dtype=np.int64)])
    tap_off = (np.arange(K)[:, None] * WP + np.arange(K)[None, :]).reshape(TAPS)

    bounds = []
    o = 0
    for csz in chunks:
        bounds.append((o, o + csz))
        o += csz

    in_maps = []
    for k in range(N_CORES):
        pix = base_pad[k * npx:(k + 1) * npx]
        idx = pix[None, :] + tap_off[:, None]          # [TAPS, npx]
        g = xpb[:, :, idx]                             # [CC, 128, TAPS, npx]
        g = g.transpose(1, 0, 2, 3)                    # [128, CC, TAPS, npx]
        xg = np.concatenate(
            [g[:, :, :, c0:c1].reshape(128, -1) for c0, c1 in bounds], axis=1)
        in_maps.append({"xg_sh": np.ascontiguousarray(xg), "w_sh": wt,
                        "b_sh": b_re})
    return in_maps, base_pad


def _unpack(results, b, mask, npx):
    b = np.asarray(b, dtype=np.float32)
    mask = np.asarray(mask)
    hs, ws = np.nonzero(mask)
    cnt = len(hs)

    out = np.empty((F, H * W), dtype=np.float32)
    out[:] = b[:, None]
    if cnt:
        y = np.concatenate(
            [results[k]["y_sh"].reshape(F, npx) for k in range(N_CORES)],
            axis=1)[:, :cnt]
        out[:, hs * W + ws] = y
    return out.reshape(1, F, H, W)


def _run(inputs, **run_kwargs):
    from concourse.bass_utils import run_bass_kernel_spmd

    mask = np.asarray(inputs["mask"])
    cnt = int((mask != 0).sum())
    npx = max(16, -(-cnt // N_CORES))
    npx = (npx + 15) // 16 * 16
    chunks = _chunks(npx)

    if npx not in _CACHE:
        _CACHE[npx] = _build(npx)
    nc = _CACHE[npx]
    in_maps, _ = _pack(inputs["x"], inputs["w"], inputs["b"], mask, npx, chunks)
    res = run_bass_kernel_spmd(nc, in_maps, core_ids=list(range(N_CORES)),
                               **run_kwargs)
    return _unpack(res.results, inputs["b"], mask, npx), res


def kernel(**inputs):
    out, _ = _run(inputs)
    return out
